# revision 6
# baseline (speedup 1.0000x reference)
"""GConvGRU (Chebyshev K=2 graph-conv GRU) + Linear head on 8 Trainium2 cores.

Strategy (graph/data parallel, per the node-partition sharding):
  - Nodes are partitioned contiguously across 8 cores (2500 rows each).
  - Only 3 sparse Laplacian applications are needed: lap(x), lap(h), lap(h*R)
    (the 6 ChebConvs share them).  Each is computed as, per destination block:
      gather source rows (dma_gather, fp16) -> one-hot matmul scatter into PSUM
    where the one-hot matrix S (norm-scaled, host-built) has edges on the
    partition (contraction) axis, so PSUM accumulates lap^T feature-major.
  - Dense gate matmuls run feature-major: out^T[mo, n] += W-blocks against
    v^T tiles, N=500 per PSUM bank; interleaved with the scatter blocks so
    the PE fills the descriptor-generation shadow of the gathers.
  - The per-(core,block) edge counts differ across cores while the NEFF is
    shared, so gather index streams are -1-padded to a uniform chunk count
    and the true per-core count is fed to dma_gather via a runtime register
    (descriptor generation on the GpSimd SWDGE is the kernel bottleneck at
    ~7.6 ns/descriptor, so padded descriptors are never generated).
  - Two NEFF launches: L1 computes lap(x), lap(h), gates Z and R, the x-part
    of the candidate pre-activation, and h*R.  The host all-gathers h*R
    (transpose + concat), then L2 computes lap(h*R), Ht, h0 and the output
    head.  Cross-partition edges are handled by replicating the (read-only)
    gather sources to every core; random connectivity makes every core need
    nearly every row anyway.
"""

import sys

sys.path.insert(0, "/opt/trn_rl_repo")

from contextlib import ExitStack
from dataclasses import dataclass

import numpy as np

import concourse.bass as bass
import concourse.tile as tile
from concourse import bacc, mybir

F16 = mybir.dt.float16
F32 = mybir.dt.float32
I16 = mybir.dt.int16
I32 = mybir.dt.int32
AF = mybir.ActivationFunctionType


@dataclass(frozen=True)
class Dims:
    N: int = 20000        # nodes
    F: int = 256          # feature dim (= IN = HID)
    LAT: int = 128        # output dim of the linear head
    CORES: int = 8
    RPC: int = 2500       # rows per core
    DBLK: int = 125       # destination-block size for the scatter
    NCH: int = 500        # dense-matmul N chunk (<=512, fits one PSUM bank)

    @property
    def NBLK(self):
        return self.RPC // self.DBLK

    @property
    def NDCH(self):
        return self.RPC // self.NCH

    @property
    def BPC(self):  # scatter blocks per dense chunk
        return self.NCH // self.DBLK

    @property
    def FH(self):
        return self.F // 2


DIMS = Dims()

# ---------------------------------------------------------------------------
# NEFF builders
# ---------------------------------------------------------------------------


def _cpb_gather(cpb):
    """Gather-side chunk counts: the first two blocks are padded to the max
    so they fully initialize both gather slots (uninitialized SBUF may hold
    NaN bit patterns that would survive the S=0 multiply; after the first
    two blocks the slots only ever hold finite gathered data)."""
    cpbmax = max(cpb)
    return tuple([cpbmax, cpbmax] + list(cpb[2:])) if len(cpb) >= 2 else cpb


class _Scatter:
    """Emits the per-dst-block gather + one-hot scatter matmuls."""

    def __init__(self, nc, tc, ctx, d, cpb, src_dram, gidx_sb, cnt_sb, S_dram,
                 lap_tiles, ew):
        self.nc, self.d, self.cpb = nc, d, cpb
        self.cpg = _cpb_gather(cpb)
        self.src_dram, self.gidx_sb, self.cnt_sb, self.S_dram = (
            src_dram, gidx_sb, cnt_sb, S_dram)
        self.lap_tiles, self.ew = lap_tiles, ew
        self.nlt = len(lap_tiles)
        assert self.nlt * 128 == ew
        self.gpool = ctx.enter_context(tc.tile_pool(name="gpool", bufs=3))
        self.spool = ctx.enter_context(tc.tile_pool(name="spool", bufs=3))
        self.lppool = ctx.enter_context(
            tc.tile_pool(name="lappsum", bufs=1, space="PSUM"))
        self.offs = np.concatenate([[0], np.cumsum(cpb)]).astype(int)
        self.goffs = np.concatenate([[0], np.cumsum(self.cpg)]).astype(int)
        # preload all per-block count registers so the gather stream never
        # interleaves with register loads
        self.regs = []
        for b in range(d.NBLK):
            creg = nc.gpsimd.alloc_register(f"cnt{b}")
            nc.gpsimd.reg_load(creg, cnt_sb[0:1, b : b + 1])
            self.regs.append(creg)

    def block(self, b):
        nc, d, cpbb, off = self.nc, self.d, self.cpb[b], int(self.offs[b])
        cpgb, goff = self.cpg[b], int(self.goffs[b])
        ew = self.ew
        G = self.gpool.tile([128, cpgb * ew], F16, tag="G", name="G")
        nc.gpsimd.dma_gather(
            out_ap=G[:].rearrange("p (c e) -> p c e", e=ew),
            in_ap=self.src_dram[:, :],
            idxs_ap=self.gidx_sb[:, 8 * goff : 8 * (goff + cpgb)],
            num_idxs=cpgb * 128,
            num_idxs_reg=self.regs[b],
            elem_size=ew,
            single_packet=False,
        )
        Sb = self.spool.tile([128, cpbb * d.DBLK], F16, tag="S", name="S")
        nc.sync.dma_start(
            out=Sb[:], in_=self.S_dram[:, d.DBLK * off : d.DBLK * (off + cpbb)])
        psums = [self.lppool.tile([128, d.DBLK], F32, tag=f"lap{i}", name=f"lp{i}")
                 for i in range(self.nlt)]
        for j in range(cpbb):
            rhs = Sb[:, j * d.DBLK : (j + 1) * d.DBLK]
            for i in range(self.nlt):
                nc.tensor.matmul(
                    out=psums[i][:],
                    lhsT=G[:, j * ew + i * 128 : j * ew + i * 128 + 128],
                    rhs=rhs,
                    start=(j == 0),
                    stop=(j == cpbb - 1),
                )
        for i, lt in enumerate(self.lap_tiles):
            nc.vector.tensor_copy(
                out=lt[:, b * d.DBLK : (b + 1) * d.DBLK], in_=psums[i][:])


def build_l1(cpb, d=DIMS):
    """L1: lap(x), lap(h); Z = sigmoid(...); R = sigmoid(...); Htx partial; hR."""
    TCH = sum(cpb)
    TCG = sum(_cpb_gather(cpb))
    nc = bacc.Bacc(
        "TRN2", target_bir_lowering=False, debug=False, enable_asserts=False,
        num_devices=d.CORES,
    )
    xh = nc.dram_tensor("xh", [d.N, 2 * d.F], F16, kind="ExternalInput").ap()
    gidx_d = nc.dram_tensor("gidx", [128, 8 * TCG], I16, kind="ExternalInput").ap()
    cnt_d = nc.dram_tensor("cnt", [1, d.NBLK], I32, kind="ExternalInput").ap()
    S_d = nc.dram_tensor("S", [128, TCH * d.DBLK], F16, kind="ExternalInput").ap()
    xT_d = nc.dram_tensor("xT", [2, 128, d.RPC], F16, kind="ExternalInput").ap()
    hT_d = nc.dram_tensor("hT", [2, 128, d.RPC], F16, kind="ExternalInput").ap()
    W_d = nc.dram_tensor("Wc", [128, 40 * 128], F16, kind="ExternalInput").ap()
    b_d = nc.dram_tensor("bias", [128, 4], F32, kind="ExternalInput").ap()
    ZT_d = nc.dram_tensor("ZT", [2, 128, d.RPC], F16, kind="ExternalOutput").ap()
    HtxT_d = nc.dram_tensor("HtxT", [2, 128, d.RPC], F32, kind="ExternalOutput").ap()
    hrT_d = nc.dram_tensor("hrT", [2, 128, d.RPC], F16, kind="ExternalOutput").ap()

    with tile.TileContext(nc) as tc, ExitStack() as ctx:
        cpool = ctx.enter_context(tc.tile_pool(name="const", bufs=1))
        gidx_sb = cpool.tile([128, 8 * TCG], I16, tag="gidx", name="gidx")
        nc.sync.dma_start(out=gidx_sb[:], in_=gidx_d[:, :])
        cnt_sb = cpool.tile([1, d.NBLK], I32, tag="cnt", name="cnt")
        nc.sync.dma_start(out=cnt_sb[:], in_=cnt_d[:, :])
        lxT_sb, lhT_sb = (
            [cpool.tile([128, d.RPC], F16, tag=f"{nm}{kh}", name=f"{nm}{kh}")
             for kh in range(2)]
            for nm in ("lxT", "lhT")
        )
        sc = _Scatter(nc, tc, ctx, d, cpb, xh, gidx_sb, cnt_sb, S_d,
                      [lxT_sb[0], lxT_sb[1], lhT_sb[0], lhT_sb[1]], 4 * 128)

        apool = ctx.enter_context(tc.tile_pool(name="acc", bufs=2, space="PSUM"))
        stg = ctx.enter_context(tc.tile_pool(name="stg", bufs=3))

        for n in range(d.NDCH):
            for b in range(n * d.BPC, (n + 1) * d.BPC):
                sc.block(b)
            if n == 0:
                # resident dense operands, behind the first blocks' S loads
                W_sb = cpool.tile([128, 40 * 128], F16, tag="W", name="W")
                nc.sync.dma_start(out=W_sb[:], in_=W_d[:, :])
                bias_sb = cpool.tile([128, 4], F32, tag="bias", name="bias")
                nc.sync.dma_start(out=bias_sb[:], in_=b_d[:, :])
                xT_sb, hT_sb = (
                    [cpool.tile([128, d.RPC], F16, tag=f"{nm}{kh}", name=f"{nm}{kh}")
                     for kh in range(2)]
                    for nm in ("xT", "hT")
                )
                for kh in range(2):
                    nc.sync.dma_start(out=xT_sb[kh][:], in_=xT_d[kh])
                    nc.sync.dma_start(out=hT_sb[kh][:], in_=hT_d[kh])
                rhs4 = [xT_sb, lxT_sb, hT_sb, lhT_sb]
            nsl = slice(n * d.NCH, (n + 1) * d.NCH)
            # gate g: 0=z (sigmoid->ZT), 1=r (sigmoid->R, then hR), 2=htx (copy)
            for g, nt in ((0, 4), (1, 4), (2, 2)):
                for mh in range(2):
                    acc = apool.tile([128, d.NCH], F32, tag="acc", name="acc")
                    k = 0
                    for t in range(nt):
                        for kh in range(2):
                            wc = g * 16 + (t * 2 + kh) * 2 + mh
                            nc.tensor.matmul(
                                out=acc[:],
                                lhsT=W_sb[:, wc * 128 : (wc + 1) * 128],
                                rhs=rhs4[t][kh][:, nsl],
                                start=(k == 0),
                                stop=(k == nt * 2 - 1),
                            )
                            k += 1
                    if g == 0:
                        zc = stg.tile([128, d.NCH], F16, tag="zc", name="zc")
                        nc.scalar.activation(out=zc[:], in_=acc[:], func=AF.Sigmoid,
                                             bias=bias_sb[:, mh : mh + 1])
                        nc.sync.dma_start(out=ZT_d[mh][:, nsl], in_=zc[:])
                    elif g == 1:
                        rc = stg.tile([128, d.NCH], F16, tag="rc", name="rc")
                        nc.scalar.activation(out=rc[:], in_=acc[:], func=AF.Sigmoid,
                                             bias=bias_sb[:, 2 + mh : 3 + mh])
                        hrc = stg.tile([128, d.NCH], F16, tag="hrc", name="hrc")
                        nc.vector.tensor_mul(out=hrc[:], in0=hT_sb[mh][:, nsl],
                                             in1=rc[:])
                        nc.sync.dma_start(out=hrT_d[mh][:, nsl], in_=hrc[:])
                    else:
                        hc = stg.tile([128, d.NCH], F32, tag="hc", name="hc")
                        nc.vector.tensor_copy(out=hc[:], in_=acc[:])
                        nc.sync.dma_start(out=HtxT_d[mh][:, nsl], in_=hc[:])
    nc.compile()
    return nc


def build_l2(cpb, d=DIMS):
    """L2: lap(hR); Ht = tanh(Htx + hR@W + lap(hR)@W + b); h0; out head."""
    TCH = sum(cpb)
    TCG = sum(_cpb_gather(cpb))
    nc = bacc.Bacc(
        "TRN2", target_bir_lowering=False, debug=False, enable_asserts=False,
        num_devices=d.CORES,
    )
    hrf = nc.dram_tensor("hrf", [d.N, d.F], F16, kind="ExternalInput").ap()
    gidx_d = nc.dram_tensor("gidx", [128, 8 * TCG], I16, kind="ExternalInput").ap()
    cnt_d = nc.dram_tensor("cnt", [1, d.NBLK], I32, kind="ExternalInput").ap()
    S_d = nc.dram_tensor("S", [128, TCH * d.DBLK], F16, kind="ExternalInput").ap()
    hrT_d = nc.dram_tensor("hrT", [2, 128, d.RPC], F16, kind="ExternalInput").ap()
    ZT_d = nc.dram_tensor("ZT", [2, 128, d.RPC], F16, kind="ExternalInput").ap()
    HtxT_d = nc.dram_tensor("HtxT", [2, 128, d.RPC], F32, kind="ExternalInput").ap()
    hT32_d = nc.dram_tensor("hT32", [2, 128, d.RPC], F32, kind="ExternalInput").ap()
    W_d = nc.dram_tensor("Wc", [128, 10 * 128], F16, kind="ExternalInput").ap()
    b_d = nc.dram_tensor("bias", [128, 3], F32, kind="ExternalInput").ap()
    h0T_d = nc.dram_tensor("h0T", [2, 128, d.RPC], F32, kind="ExternalOutput").ap()
    outT_d = nc.dram_tensor("outT", [128, d.RPC], F32, kind="ExternalOutput").ap()

    with tile.TileContext(nc) as tc, ExitStack() as ctx:
        cpool = ctx.enter_context(tc.tile_pool(name="const", bufs=1))
        gidx_sb = cpool.tile([128, 8 * TCG], I16, tag="gidx", name="gidx")
        nc.sync.dma_start(out=gidx_sb[:], in_=gidx_d[:, :])
        cnt_sb = cpool.tile([1, d.NBLK], I32, tag="cnt", name="cnt")
        nc.sync.dma_start(out=cnt_sb[:], in_=cnt_d[:, :])
        lhrT_sb = [cpool.tile([128, d.RPC], F16, tag=f"lhrT{kh}", name=f"lhrT{kh}")
                   for kh in range(2)]
        sc = _Scatter(nc, tc, ctx, d, cpb, hrf, gidx_sb, cnt_sb, S_d,
                      [lhrT_sb[0], lhrT_sb[1]], 2 * 128)

        apool = ctx.enter_context(tc.tile_pool(name="acc", bufs=2, space="PSUM"))
        stg = ctx.enter_context(tc.tile_pool(name="stg", bufs=3))

        for n in range(d.NDCH):
            for b in range(n * d.BPC, (n + 1) * d.BPC):
                sc.block(b)
            if n == 0:
                W_sb = cpool.tile([128, 10 * 128], F16, tag="W", name="W")
                nc.sync.dma_start(out=W_sb[:], in_=W_d[:, :])
                bias_sb = cpool.tile([128, 3], F32, tag="bias", name="bias")
                nc.sync.dma_start(out=bias_sb[:], in_=b_d[:, :])
                hrT_sb, ZT_sb = (
                    [cpool.tile([128, d.RPC], F16, tag=f"{nm}{kh}", name=f"{nm}{kh}")
                     for kh in range(2)]
                    for nm in ("hrT", "ZT")
                )
                HtxT_sb, hT32_sb = (
                    [cpool.tile([128, d.RPC], F32, tag=f"{nm}{kh}", name=f"{nm}{kh}")
                     for kh in range(2)]
                    for nm in ("HtxT", "hT32")
                )
                for kh in range(2):
                    nc.sync.dma_start(out=hrT_sb[kh][:], in_=hrT_d[kh])
                    nc.sync.dma_start(out=ZT_sb[kh][:], in_=ZT_d[kh])
                    nc.sync.dma_start(out=HtxT_sb[kh][:], in_=HtxT_d[kh])
                    nc.sync.dma_start(out=hT32_sb[kh][:], in_=hT32_d[kh])
                rhs2 = [hrT_sb, lhrT_sb]
            nsl = slice(n * d.NCH, (n + 1) * d.NCH)
            relu_c = []
            for mh in range(2):
                acc = apool.tile([128, d.NCH], F32, tag="acc", name="acc")
                k = 0
                for t in range(2):
                    for kh in range(2):
                        wc = (t * 2 + kh) * 2 + mh
                        nc.tensor.matmul(
                            out=acc[:],
                            lhsT=W_sb[:, wc * 128 : (wc + 1) * 128],
                            rhs=rhs2[t][kh][:, nsl],
                            start=(k == 0), stop=(k == 3),
                        )
                        k += 1
                nc.vector.tensor_add(out=acc[:], in0=acc[:],
                                     in1=HtxT_sb[mh][:, nsl])
                ht = stg.tile([128, d.NCH], F32, tag="ht", name="ht")
                nc.scalar.activation(out=ht[:], in_=acc[:], func=AF.Tanh,
                                     bias=bias_sb[:, mh : mh + 1])
                # h0 = Ht + Z*(h - Ht)
                z32 = stg.tile([128, d.NCH], F32, tag="z32", name="z32")
                nc.vector.tensor_copy(out=z32[:], in_=ZT_sb[mh][:, nsl])
                dd = stg.tile([128, d.NCH], F32, tag="dd", name="dd")
                nc.vector.tensor_sub(out=dd[:], in0=hT32_sb[mh][:, nsl], in1=ht[:])
                nc.vector.tensor_mul(out=dd[:], in0=dd[:], in1=z32[:])
                h0c = stg.tile([128, d.NCH], F32, tag="h0c", name="h0c")
                nc.vector.tensor_add(out=h0c[:], in0=ht[:], in1=dd[:])
                nc.sync.dma_start(out=h0T_d[mh][:, nsl], in_=h0c[:])
                rl = stg.tile([128, d.NCH], F16, tag="rl", name="rl")
                nc.vector.tensor_scalar_max(rl[:], h0c[:], 0.0)
                relu_c.append(rl)
            acc = apool.tile([128, d.NCH], F32, tag="acc", name="acc")
            for kh in range(2):
                nc.tensor.matmul(
                    out=acc[:], lhsT=W_sb[:, (8 + kh) * 128 : (9 + kh) * 128],
                    rhs=relu_c[kh][:], start=(kh == 0), stop=(kh == 1),
                )
            oc = stg.tile([128, d.NCH], F32, tag="oc", name="oc")
            nc.vector.tensor_scalar_add(oc[:], acc[:], bias_sb[:, 2:3])
            nc.sync.dma_start(out=outT_d[:, nsl], in_=oc[:])
    nc.compile()
    return nc


# ---------------------------------------------------------------------------
# Host-side preprocessing
# ---------------------------------------------------------------------------


def _wrap_idx(idx_arr):
    """Edge-index array -> dma_gather wrapped layout [128, len/16] int16."""
    a = idx_arr.reshape(-1, 16).T  # [16, n/16]
    return np.tile(a, (8, 1))


def prep_edges(edge_index, edge_weight, d=DIMS):
    """Partition/sort/pad edges; build per-core gather-index + scatter tensors."""
    src = edge_index[0].astype(np.int64)
    dst = edge_index[1].astype(np.int64)
    w = edge_weight.astype(np.float32)
    deg = np.zeros(d.N, np.float32)
    np.add.at(deg, src, w)
    dis = np.where(deg > 0, 1.0 / np.sqrt(np.where(deg > 0, deg, 1.0)), 0.0).astype(
        np.float32
    )
    norm = (-dis[src] * w * dis[dst]).astype(np.float32)

    core = dst // d.RPC
    blk = (dst % d.RPC) // d.DBLK
    dloc = (dst % d.RPC) % d.DBLK
    order = np.lexsort((src, blk, core))
    src_s, norm_s, dloc_s = src[order], norm[order], dloc[order]
    counts = np.bincount(core * d.NBLK + blk, minlength=d.CORES * d.NBLK).reshape(
        d.CORES, d.NBLK
    )
    counts = np.maximum(counts, 1)  # dma_gather needs >=1 valid index
    cpb = tuple(
        max(1, int(np.ceil(counts[:, b].max() / 128))) for b in range(d.NBLK)
    )
    TCH = sum(cpb)
    real = np.bincount(core * d.NBLK + blk, minlength=d.CORES * d.NBLK).reshape(
        d.CORES, d.NBLK
    )
    starts = np.zeros(d.CORES * d.NBLK + 1, np.int64)
    np.cumsum(real.reshape(-1), out=starts[1:])

    cpg = _cpb_gather(cpb)
    TCG = sum(cpg)
    gidx = np.full((d.CORES, 128, 8 * TCG), -1, np.int16)
    S = np.zeros((d.CORES, 128, TCH * d.DBLK), np.float16)
    cnt_out = np.zeros((d.CORES, d.NBLK), np.int32)
    for c in range(d.CORES):
        off = goff = 0
        for b in range(d.NBLK):
            cnt = real[c, b]
            s0 = starts[c * d.NBLK + b]
            if b < 2:
                # first two blocks fully initialize the gather slots: pad
                # with valid index 0 (S stays 0 there) and gather everything
                idx_arr = np.zeros(cpg[b] * 128, np.int16)
                cnt_out[c, b] = cpg[b] * 128
            else:
                idx_arr = np.full(cpg[b] * 128, -1, np.int16)
                cnt_out[c, b] = max(cnt, 1)
                if cnt == 0:
                    idx_arr[0] = 0  # keep >=1 valid index (S row stays 0)
            idx_arr[:cnt] = src_s[s0 : s0 + cnt]
            gidx[c, :, 8 * goff : 8 * (goff + cpg[b])] = _wrap_idx(idx_arr)
            p = np.arange(cnt)
            S[c, p % 128, d.DBLK * off + (p // 128) * d.DBLK + dloc_s[s0 : s0 + cnt]] = (
                norm_s[s0 : s0 + cnt]
            )
            off += cpb[b]
            goff += cpg[b]
    return cpb, gidx, S, cnt_out


def _pack_w(mats):
    """mats: list of [256,256]-ish fp32 -> [128, nblocks*128] fp16 lhsT blocks."""
    blocks = []
    for m in mats:
        K, M = m.shape
        for kh in range(K // 128):
            for mh in range(M // 128):
                blocks.append(m[kh * 128 : (kh + 1) * 128, mh * 128 : (mh + 1) * 128])
    return np.concatenate(blocks, axis=1).astype(np.float16)


def _halves(vT):
    """[F, R] -> [2, 128, R] contiguous."""
    F = vT.shape[0]
    return np.ascontiguousarray(vT.reshape(2, F // 2, -1))


# ---------------------------------------------------------------------------
# kernel() entry point
# ---------------------------------------------------------------------------

_CACHE = {}
LAST_EXEC_NS = []


def _get_neffs(cpb, d):
    key = (cpb, d)
    if key not in _CACHE:
        _CACHE[key] = (build_l1(cpb, d), build_l2(cpb, d))
    return _CACHE[key]


def _run(nc, in_maps, d):
    from concourse.bass_utils import run_bass_kernel_spmd

    res = run_bass_kernel_spmd(nc, in_maps, core_ids=list(range(d.CORES)))
    if res.exec_time_ns is not None:
        LAST_EXEC_NS.append(res.exec_time_ns)
    return res.results


def kernel(
    x, h, edge_index, edge_weight,
    Wxz, bxz, Whz, bhz, Wxr, bxr, Whr, bhr, Wxh, bxh, Whh, bhh, Wl, bl,
    dims=DIMS,
):
    d = dims
    LAST_EXEC_NS.clear()
    x = np.asarray(x, np.float32)
    h = np.asarray(h, np.float32)
    cpb, gidx, S, counts = prep_edges(
        np.asarray(edge_index), np.asarray(edge_weight), d)
    nc1, nc2 = _get_neffs(cpb, d)

    xh = np.concatenate([x, h], axis=1).astype(np.float16)
    xT = np.ascontiguousarray(x.T)
    hT = np.ascontiguousarray(h.T)

    W1 = _pack_w([
        np.asarray(Wxz[0]), np.asarray(Wxz[1]), np.asarray(Whz[0]), np.asarray(Whz[1]),
        np.asarray(Wxr[0]), np.asarray(Wxr[1]), np.asarray(Whr[0]), np.asarray(Whr[1]),
        np.asarray(Wxh[0]), np.asarray(Wxh[1]),
    ])
    bz = (np.asarray(bxz) + np.asarray(bhz)).astype(np.float32)
    br = (np.asarray(bxr) + np.asarray(bhr)).astype(np.float32)
    bias1 = np.stack(
        [bz[: d.FH], bz[d.FH :], br[: d.FH], br[d.FH :]], axis=1
    ).astype(np.float32)

    in1 = []
    for c in range(d.CORES):
        rs = slice(c * d.RPC, (c + 1) * d.RPC)
        in1.append({
            "xh": xh,
            "gidx": np.ascontiguousarray(gidx[c]),
            "cnt": counts[c : c + 1],
            "S": np.ascontiguousarray(S[c]),
            "xT": _halves(xT[:, rs]).astype(np.float16),
            "hT": _halves(hT[:, rs]).astype(np.float16),
            "Wc": W1,
            "bias": bias1,
        })
    res1 = _run(nc1, in1, d)

    # host exchange: assemble full h*R, row-major fp16
    hr_full = np.empty((d.N, d.F), np.float16)
    for c in range(d.CORES):
        hrT_c = res1[c]["hrT"]  # [2, 128, RPC]
        rs = slice(c * d.RPC, (c + 1) * d.RPC)
        hr_full[rs, : d.FH] = hrT_c[0].T
        hr_full[rs, d.FH :] = hrT_c[1].T

    W2 = _pack_w([np.asarray(Whh[0]), np.asarray(Whh[1]), np.asarray(Wl)])
    bht = (np.asarray(bxh) + np.asarray(bhh)).astype(np.float32)
    bias2 = np.stack(
        [bht[: d.FH], bht[d.FH :], np.asarray(bl, np.float32)], axis=1
    ).astype(np.float32)

    in2 = []
    for c in range(d.CORES):
        rs = slice(c * d.RPC, (c + 1) * d.RPC)
        in2.append({
            "hrf": hr_full,
            "gidx": np.ascontiguousarray(gidx[c]),
            "cnt": counts[c : c + 1],
            "S": np.ascontiguousarray(S[c]),
            "hrT": res1[c]["hrT"],
            "ZT": res1[c]["ZT"],
            "HtxT": res1[c]["HtxT"],
            "hT32": _halves(hT[:, rs]).astype(np.float32),
            "Wc": W2,
            "bias": bias2,
        })
    res2 = _run(nc2, in2, d)

    out = np.empty((d.N, d.LAT), np.float32)
    h0 = np.empty((d.N, d.F), np.float32)
    for c in range(d.CORES):
        rs = slice(c * d.RPC, (c + 1) * d.RPC)
        out[rs] = res2[c]["outT"].T
        h0[rs, : d.FH] = res2[c]["h0T"][0].T
        h0[rs, d.FH :] = res2[c]["h0T"][1].T
    return out, h0


# revision 7
# speedup vs baseline: 1.0424x; 1.0424x over previous
"""GConvGRU (Chebyshev K=2 graph-conv GRU) + Linear head on 8 Trainium2 cores.

Strategy (graph/data parallel, per the node-partition sharding):
  - Nodes are partitioned contiguously across 8 cores (2500 rows each).
  - Only 3 sparse Laplacian applications are needed: lap(x), lap(h), lap(h*R)
    (the 6 ChebConvs share them).  Each is computed as, per destination block:
      gather source rows (dma_gather, fp16) -> one-hot matmul scatter into PSUM
    where the one-hot matrix S (norm-scaled, host-built) has edges on the
    partition (contraction) axis, so PSUM accumulates lap^T feature-major.
    x and h are interleaved per row so one descriptor fetches both.
  - Dense gate matmuls run feature-major: out^T[mo, n] += W-blocks against
    v^T tiles, N=500 per PSUM bank; interleaved with the scatter blocks so
    the PE fills the descriptor-generation shadow of the gathers (SWDGE
    descriptor generation, ~7.6 ns/edge on GpSimd, is the kernel bottleneck;
    everything else is scheduled to hide underneath it).
  - Per-(core,block) edge counts differ across cores while the NEFF is
    shared, so index streams are padded to a uniform chunk count with
    index 0 whose scatter coefficients are zero.
  - Two NEFF launches: L1 computes lap(x), lap(h), gates Z and R, the x-part
    of the candidate pre-activation, and h*R.  The host all-gathers h*R
    (transpose + concat), then L2 computes lap(h*R), Ht, h0 and the output
    head.  Cross-partition edges are handled by replicating the (read-only)
    gather sources to every core; random connectivity makes every core need
    nearly every row anyway.
"""

import sys

sys.path.insert(0, "/opt/trn_rl_repo")

from contextlib import ExitStack
from dataclasses import dataclass

import numpy as np

import concourse.bass as bass
import concourse.tile as tile
from concourse import bacc, mybir

F16 = mybir.dt.float16
F32 = mybir.dt.float32
I16 = mybir.dt.int16
AF = mybir.ActivationFunctionType


@dataclass(frozen=True)
class Dims:
    N: int = 20000        # nodes
    F: int = 256          # feature dim (= IN = HID)
    LAT: int = 128        # output dim of the linear head
    CORES: int = 8
    RPC: int = 2500       # rows per core
    DBLK: int = 125       # destination-block size for the scatter
    NCH: int = 500        # dense-matmul N chunk (<=512, fits one PSUM bank)

    @property
    def NBLK(self):
        return self.RPC // self.DBLK

    @property
    def NDCH(self):
        return self.RPC // self.NCH

    @property
    def BPC(self):  # scatter blocks per dense chunk
        return self.NCH // self.DBLK

    @property
    def FH(self):
        return self.F // 2


DIMS = Dims()

# ---------------------------------------------------------------------------
# NEFF builders
# ---------------------------------------------------------------------------


class _Scatter:
    """Emits the per-dst-block gather + one-hot scatter matmuls."""

    def __init__(self, nc, tc, ctx, d, cpb, src_dram, gidx_sb, S_dram,
                 lap_tiles, ew):
        self.nc, self.d, self.cpb = nc, d, cpb
        self.src_dram, self.gidx_sb, self.S_dram = src_dram, gidx_sb, S_dram
        self.lap_tiles, self.ew = lap_tiles, ew
        self.nlt = len(lap_tiles)
        assert self.nlt * 128 == ew
        self.gpool = ctx.enter_context(tc.tile_pool(name="gpool", bufs=3))
        self.spool = ctx.enter_context(tc.tile_pool(name="spool", bufs=3))
        self.lppool = ctx.enter_context(
            tc.tile_pool(name="lappsum", bufs=1, space="PSUM"))
        self.offs = np.concatenate([[0], np.cumsum(cpb)]).astype(int)

    def block(self, b):
        nc, d, cpbb, off = self.nc, self.d, self.cpb[b], int(self.offs[b])
        ew = self.ew
        G = self.gpool.tile([128, cpbb * ew], F16, tag="G", name="G")
        nc.gpsimd.dma_gather(
            out_ap=G[:].rearrange("p (c e) -> p c e", e=ew),
            in_ap=self.src_dram[:, :],
            idxs_ap=self.gidx_sb[:, 8 * off : 8 * (off + cpbb)],
            num_idxs=cpbb * 128,
            num_idxs_reg=cpbb * 128,
            elem_size=ew,
            single_packet=False,
        )
        Sb = self.spool.tile([128, cpbb * d.DBLK], F16, tag="S", name="S")
        nc.sync.dma_start(
            out=Sb[:], in_=self.S_dram[:, d.DBLK * off : d.DBLK * (off + cpbb)])
        psums = [self.lppool.tile([128, d.DBLK], F32, tag=f"lap{i}", name=f"lp{i}")
                 for i in range(self.nlt)]
        for j in range(cpbb):
            rhs = Sb[:, j * d.DBLK : (j + 1) * d.DBLK]
            for i in range(self.nlt):
                nc.tensor.matmul(
                    out=psums[i][:],
                    lhsT=G[:, j * ew + i * 128 : j * ew + i * 128 + 128],
                    rhs=rhs,
                    start=(j == 0),
                    stop=(j == cpbb - 1),
                )
        for i, lt in enumerate(self.lap_tiles):
            nc.vector.tensor_copy(
                out=lt[:, b * d.DBLK : (b + 1) * d.DBLK], in_=psums[i][:])


def build_l1(cpb, d=DIMS):
    """L1: lap(x), lap(h); Z = sigmoid(...); R = sigmoid(...); Htx partial; hR."""
    TCH = sum(cpb)
    nc = bacc.Bacc(
        "TRN2", target_bir_lowering=False, debug=False, enable_asserts=False,
        num_devices=d.CORES,
    )
    xh = nc.dram_tensor("xh", [d.N, 2 * d.F], F16, kind="ExternalInput").ap()
    gidx_d = nc.dram_tensor("gidx", [128, 8 * TCH], I16, kind="ExternalInput").ap()
    S_d = nc.dram_tensor("S", [128, TCH * d.DBLK], F16, kind="ExternalInput").ap()
    xT_d = nc.dram_tensor("xT", [2, 128, d.RPC], F16, kind="ExternalInput").ap()
    hT_d = nc.dram_tensor("hT", [2, 128, d.RPC], F16, kind="ExternalInput").ap()
    W_d = nc.dram_tensor("Wc", [128, 40 * 128], F16, kind="ExternalInput").ap()
    b_d = nc.dram_tensor("bias", [128, 4], F32, kind="ExternalInput").ap()
    ZT_d = nc.dram_tensor("ZT", [2, 128, d.RPC], F16, kind="ExternalOutput").ap()
    HtxT_d = nc.dram_tensor("HtxT", [2, 128, d.RPC], F32, kind="ExternalOutput").ap()
    hrT_d = nc.dram_tensor("hrT", [2, 128, d.RPC], F16, kind="ExternalOutput").ap()

    with tile.TileContext(nc) as tc, ExitStack() as ctx:
        cpool = ctx.enter_context(tc.tile_pool(name="const", bufs=1))
        # gidx rides the gather engine's own SWDGE queue so the first
        # dma_gather doesn't wait behind the big HWDGE input batch
        gidx_sb = cpool.tile([128, 8 * TCH], I16, tag="gidx", name="gidx")
        nc.gpsimd.dma_start(out=gidx_sb[:], in_=gidx_d[:, :])
        lxT_sb, lhT_sb = (
            [cpool.tile([128, d.RPC], F16, tag=f"{nm}{kh}", name=f"{nm}{kh}")
             for kh in range(2)]
            for nm in ("lxT", "lhT")
        )
        sc = _Scatter(nc, tc, ctx, d, cpb, xh, gidx_sb, S_d,
                      [lxT_sb[0], lxT_sb[1], lhT_sb[0], lhT_sb[1]], 4 * 128)

        apool = ctx.enter_context(tc.tile_pool(name="acc", bufs=3, space="PSUM"))
        stg = ctx.enter_context(tc.tile_pool(name="stg", bufs=3))

        for n in range(d.NDCH):
            for b in range(n * d.BPC, (n + 1) * d.BPC):
                sc.block(b)
            if n == 0:
                # resident dense operands, behind the first blocks' S loads
                W_sb = cpool.tile([128, 40 * 128], F16, tag="W", name="W")
                nc.sync.dma_start(out=W_sb[:], in_=W_d[:, :])
                bias_sb = cpool.tile([128, 4], F32, tag="bias", name="bias")
                nc.sync.dma_start(out=bias_sb[:], in_=b_d[:, :])
                xT_sb, hT_sb = (
                    [cpool.tile([128, d.RPC], F16, tag=f"{nm}{kh}", name=f"{nm}{kh}")
                     for kh in range(2)]
                    for nm in ("xT", "hT")
                )
                for kh in range(2):
                    nc.sync.dma_start(out=xT_sb[kh][:], in_=xT_d[kh])
                    nc.sync.dma_start(out=hT_sb[kh][:], in_=hT_d[kh])
                rhs4 = [xT_sb, lxT_sb, hT_sb, lhT_sb]
            nsl = slice(n * d.NCH, (n + 1) * d.NCH)
            # gate g: 0=z (sigmoid->ZT), 1=r (sigmoid->R, then hR), 2=htx (copy)
            for g, nt in ((0, 4), (1, 4), (2, 2)):
                for mh in range(2):
                    acc = apool.tile([128, d.NCH], F32, tag="acc", name="acc")
                    k = 0
                    for t in range(nt):
                        for kh in range(2):
                            wc = g * 16 + (t * 2 + kh) * 2 + mh
                            nc.tensor.matmul(
                                out=acc[:],
                                lhsT=W_sb[:, wc * 128 : (wc + 1) * 128],
                                rhs=rhs4[t][kh][:, nsl],
                                start=(k == 0),
                                stop=(k == nt * 2 - 1),
                            )
                            k += 1
                    if g == 0:
                        zc = stg.tile([128, d.NCH], F16, tag="zc", name="zc")
                        nc.scalar.activation(out=zc[:], in_=acc[:], func=AF.Sigmoid,
                                             bias=bias_sb[:, mh : mh + 1])
                        nc.sync.dma_start(out=ZT_d[mh][:, nsl], in_=zc[:])
                    elif g == 1:
                        rc = stg.tile([128, d.NCH], F16, tag="rc", name="rc")
                        nc.scalar.activation(out=rc[:], in_=acc[:], func=AF.Sigmoid,
                                             bias=bias_sb[:, 2 + mh : 3 + mh])
                        hrc = stg.tile([128, d.NCH], F16, tag="hrc", name="hrc")
                        nc.vector.tensor_mul(out=hrc[:], in0=hT_sb[mh][:, nsl],
                                             in1=rc[:])
                        nc.sync.dma_start(out=hrT_d[mh][:, nsl], in_=hrc[:])
                    else:
                        hc = stg.tile([128, d.NCH], F32, tag="hc", name="hc")
                        nc.vector.tensor_copy(out=hc[:], in_=acc[:])
                        nc.sync.dma_start(out=HtxT_d[mh][:, nsl], in_=hc[:])
    nc.compile()
    return nc


def build_l2(cpb, d=DIMS):
    """L2: lap(hR); Ht = tanh(Htx + hR@W + lap(hR)@W + b); h0; out head."""
    TCH = sum(cpb)
    nc = bacc.Bacc(
        "TRN2", target_bir_lowering=False, debug=False, enable_asserts=False,
        num_devices=d.CORES,
    )
    hrf = nc.dram_tensor("hrf", [d.N, d.F], F16, kind="ExternalInput").ap()
    gidx_d = nc.dram_tensor("gidx", [128, 8 * TCH], I16, kind="ExternalInput").ap()
    S_d = nc.dram_tensor("S", [128, TCH * d.DBLK], F16, kind="ExternalInput").ap()
    hrT_d = nc.dram_tensor("hrT", [2, 128, d.RPC], F16, kind="ExternalInput").ap()
    ZT_d = nc.dram_tensor("ZT", [2, 128, d.RPC], F16, kind="ExternalInput").ap()
    HtxT_d = nc.dram_tensor("HtxT", [2, 128, d.RPC], F32, kind="ExternalInput").ap()
    hT32_d = nc.dram_tensor("hT32", [2, 128, d.RPC], F32, kind="ExternalInput").ap()
    W_d = nc.dram_tensor("Wc", [128, 10 * 128], F16, kind="ExternalInput").ap()
    b_d = nc.dram_tensor("bias", [128, 3], F32, kind="ExternalInput").ap()
    h0T_d = nc.dram_tensor("h0T", [2, 128, d.RPC], F32, kind="ExternalOutput").ap()
    outT_d = nc.dram_tensor("outT", [128, d.RPC], F32, kind="ExternalOutput").ap()

    with tile.TileContext(nc) as tc, ExitStack() as ctx:
        cpool = ctx.enter_context(tc.tile_pool(name="const", bufs=1))
        gidx_sb = cpool.tile([128, 8 * TCH], I16, tag="gidx", name="gidx")
        nc.gpsimd.dma_start(out=gidx_sb[:], in_=gidx_d[:, :])
        lhrT_sb = [cpool.tile([128, d.RPC], F16, tag=f"lhrT{kh}", name=f"lhrT{kh}")
                   for kh in range(2)]
        sc = _Scatter(nc, tc, ctx, d, cpb, hrf, gidx_sb, S_d,
                      [lhrT_sb[0], lhrT_sb[1]], 2 * 128)

        apool = ctx.enter_context(tc.tile_pool(name="acc", bufs=3, space="PSUM"))
        stg = ctx.enter_context(tc.tile_pool(name="stg", bufs=3))

        for n in range(d.NDCH):
            for b in range(n * d.BPC, (n + 1) * d.BPC):
                sc.block(b)
            if n == 0:
                W_sb = cpool.tile([128, 10 * 128], F16, tag="W", name="W")
                nc.sync.dma_start(out=W_sb[:], in_=W_d[:, :])
                bias_sb = cpool.tile([128, 3], F32, tag="bias", name="bias")
                nc.sync.dma_start(out=bias_sb[:], in_=b_d[:, :])
                hrT_sb, ZT_sb = (
                    [cpool.tile([128, d.RPC], F16, tag=f"{nm}{kh}", name=f"{nm}{kh}")
                     for kh in range(2)]
                    for nm in ("hrT", "ZT")
                )
                HtxT_sb, hT32_sb = (
                    [cpool.tile([128, d.RPC], F32, tag=f"{nm}{kh}", name=f"{nm}{kh}")
                     for kh in range(2)]
                    for nm in ("HtxT", "hT32")
                )
                for kh in range(2):
                    nc.sync.dma_start(out=hrT_sb[kh][:], in_=hrT_d[kh])
                    nc.sync.dma_start(out=ZT_sb[kh][:], in_=ZT_d[kh])
                    nc.sync.dma_start(out=HtxT_sb[kh][:], in_=HtxT_d[kh])
                    nc.sync.dma_start(out=hT32_sb[kh][:], in_=hT32_d[kh])
                rhs2 = [hrT_sb, lhrT_sb]
            nsl = slice(n * d.NCH, (n + 1) * d.NCH)
            relu_c = []
            for mh in range(2):
                acc = apool.tile([128, d.NCH], F32, tag="acc", name="acc")
                k = 0
                for t in range(2):
                    for kh in range(2):
                        wc = (t * 2 + kh) * 2 + mh
                        nc.tensor.matmul(
                            out=acc[:],
                            lhsT=W_sb[:, wc * 128 : (wc + 1) * 128],
                            rhs=rhs2[t][kh][:, nsl],
                            start=(k == 0), stop=(k == 3),
                        )
                        k += 1
                nc.vector.tensor_add(out=acc[:], in0=acc[:],
                                     in1=HtxT_sb[mh][:, nsl])
                ht = stg.tile([128, d.NCH], F32, tag="ht", name="ht")
                nc.scalar.activation(out=ht[:], in_=acc[:], func=AF.Tanh,
                                     bias=bias_sb[:, mh : mh + 1])
                # h0 = Ht + Z*(h - Ht)
                z32 = stg.tile([128, d.NCH], F32, tag="z32", name="z32")
                nc.vector.tensor_copy(out=z32[:], in_=ZT_sb[mh][:, nsl])
                dd = stg.tile([128, d.NCH], F32, tag="dd", name="dd")
                nc.vector.tensor_sub(out=dd[:], in0=hT32_sb[mh][:, nsl], in1=ht[:])
                nc.vector.tensor_mul(out=dd[:], in0=dd[:], in1=z32[:])
                h0c = stg.tile([128, d.NCH], F32, tag="h0c", name="h0c")
                nc.vector.tensor_add(out=h0c[:], in0=ht[:], in1=dd[:])
                nc.sync.dma_start(out=h0T_d[mh][:, nsl], in_=h0c[:])
                rl = stg.tile([128, d.NCH], F16, tag="rl", name="rl")
                nc.vector.tensor_scalar_max(rl[:], h0c[:], 0.0)
                relu_c.append(rl)
            acc = apool.tile([128, d.NCH], F32, tag="acc", name="acc")
            for kh in range(2):
                nc.tensor.matmul(
                    out=acc[:], lhsT=W_sb[:, (8 + kh) * 128 : (9 + kh) * 128],
                    rhs=relu_c[kh][:], start=(kh == 0), stop=(kh == 1),
                )
            oc = stg.tile([128, d.NCH], F32, tag="oc", name="oc")
            nc.vector.tensor_scalar_add(oc[:], acc[:], bias_sb[:, 2:3])
            nc.sync.dma_start(out=outT_d[:, nsl], in_=oc[:])
    nc.compile()
    return nc


# ---------------------------------------------------------------------------
# Host-side preprocessing
# ---------------------------------------------------------------------------


def _wrap_idx(idx_arr):
    """Edge-index array -> dma_gather wrapped layout [128, len/16] int16."""
    a = idx_arr.reshape(-1, 16).T  # [16, n/16]
    return np.tile(a, (8, 1))


def prep_edges(edge_index, edge_weight, d=DIMS):
    """Partition/sort/pad edges; build per-core gather-index + scatter tensors."""
    src = edge_index[0].astype(np.int64)
    dst = edge_index[1].astype(np.int64)
    w = edge_weight.astype(np.float32)
    deg = np.zeros(d.N, np.float32)
    np.add.at(deg, src, w)
    dis = np.where(deg > 0, 1.0 / np.sqrt(np.where(deg > 0, deg, 1.0)), 0.0).astype(
        np.float32
    )
    norm = (-dis[src] * w * dis[dst]).astype(np.float32)

    core = dst // d.RPC
    blk = (dst % d.RPC) // d.DBLK
    dloc = (dst % d.RPC) % d.DBLK
    order = np.lexsort((src, blk, core))
    src_s, norm_s, dloc_s = src[order], norm[order], dloc[order]
    counts = np.bincount(core * d.NBLK + blk, minlength=d.CORES * d.NBLK).reshape(
        d.CORES, d.NBLK
    )
    cpb = tuple(
        max(1, int(np.ceil(counts[:, b].max() / 128))) for b in range(d.NBLK)
    )
    TCH = sum(cpb)
    starts = np.zeros(d.CORES * d.NBLK + 1, np.int64)
    np.cumsum(counts.reshape(-1), out=starts[1:])

    # pad with index 0 (valid row; its scatter coefficients stay 0)
    gidx = np.zeros((d.CORES, 128, 8 * TCH), np.int16)
    S = np.zeros((d.CORES, 128, TCH * d.DBLK), np.float16)
    for c in range(d.CORES):
        off = 0
        for b in range(d.NBLK):
            cnt = counts[c, b]
            s0 = starts[c * d.NBLK + b]
            idx_arr = np.zeros(cpb[b] * 128, np.int16)
            idx_arr[:cnt] = src_s[s0 : s0 + cnt]
            gidx[c, :, 8 * off : 8 * (off + cpb[b])] = _wrap_idx(idx_arr)
            p = np.arange(cnt)
            S[c, p % 128, d.DBLK * off + (p // 128) * d.DBLK + dloc_s[s0 : s0 + cnt]] = (
                norm_s[s0 : s0 + cnt]
            )
            off += cpb[b]
    return cpb, gidx, S


def _pack_w(mats):
    """mats: list of [256,256]-ish fp32 -> [128, nblocks*128] fp16 lhsT blocks."""
    blocks = []
    for m in mats:
        K, M = m.shape
        for kh in range(K // 128):
            for mh in range(M // 128):
                blocks.append(m[kh * 128 : (kh + 1) * 128, mh * 128 : (mh + 1) * 128])
    return np.concatenate(blocks, axis=1).astype(np.float16)


def _halves(vT):
    """[F, R] -> [2, 128, R] contiguous."""
    F = vT.shape[0]
    return np.ascontiguousarray(vT.reshape(2, F // 2, -1))


# ---------------------------------------------------------------------------
# kernel() entry point
# ---------------------------------------------------------------------------

_CACHE = {}
LAST_EXEC_NS = []


def _get_neffs(cpb, d):
    key = (cpb, d)
    if key not in _CACHE:
        _CACHE[key] = (build_l1(cpb, d), build_l2(cpb, d))
    return _CACHE[key]


def _run(nc, in_maps, d):
    from concourse.bass_utils import run_bass_kernel_spmd

    res = run_bass_kernel_spmd(nc, in_maps, core_ids=list(range(d.CORES)))
    if res.exec_time_ns is not None:
        LAST_EXEC_NS.append(res.exec_time_ns)
    return res.results


def kernel(
    x, h, edge_index, edge_weight,
    Wxz, bxz, Whz, bhz, Wxr, bxr, Whr, bhr, Wxh, bxh, Whh, bhh, Wl, bl,
    dims=DIMS,
):
    d = dims
    LAST_EXEC_NS.clear()
    x = np.asarray(x, np.float32)
    h = np.asarray(h, np.float32)
    cpb, gidx, S = prep_edges(np.asarray(edge_index), np.asarray(edge_weight), d)
    nc1, nc2 = _get_neffs(cpb, d)

    xh = np.concatenate([x, h], axis=1).astype(np.float16)
    xT = np.ascontiguousarray(x.T)
    hT = np.ascontiguousarray(h.T)

    W1 = _pack_w([
        np.asarray(Wxz[0]), np.asarray(Wxz[1]), np.asarray(Whz[0]), np.asarray(Whz[1]),
        np.asarray(Wxr[0]), np.asarray(Wxr[1]), np.asarray(Whr[0]), np.asarray(Whr[1]),
        np.asarray(Wxh[0]), np.asarray(Wxh[1]),
    ])
    bz = (np.asarray(bxz) + np.asarray(bhz)).astype(np.float32)
    br = (np.asarray(bxr) + np.asarray(bhr)).astype(np.float32)
    bias1 = np.stack(
        [bz[: d.FH], bz[d.FH :], br[: d.FH], br[d.FH :]], axis=1
    ).astype(np.float32)

    in1 = []
    for c in range(d.CORES):
        rs = slice(c * d.RPC, (c + 1) * d.RPC)
        in1.append({
            "xh": xh,
            "gidx": np.ascontiguousarray(gidx[c]),
            "S": np.ascontiguousarray(S[c]),
            "xT": _halves(xT[:, rs]).astype(np.float16),
            "hT": _halves(hT[:, rs]).astype(np.float16),
            "Wc": W1,
            "bias": bias1,
        })
    res1 = _run(nc1, in1, d)

    # host exchange: assemble full h*R, row-major fp16
    hr_full = np.empty((d.N, d.F), np.float16)
    for c in range(d.CORES):
        hrT_c = res1[c]["hrT"]  # [2, 128, RPC]
        rs = slice(c * d.RPC, (c + 1) * d.RPC)
        hr_full[rs, : d.FH] = hrT_c[0].T
        hr_full[rs, d.FH :] = hrT_c[1].T

    W2 = _pack_w([np.asarray(Whh[0]), np.asarray(Whh[1]), np.asarray(Wl)])
    bht = (np.asarray(bxh) + np.asarray(bhh)).astype(np.float32)
    bias2 = np.stack(
        [bht[: d.FH], bht[d.FH :], np.asarray(bl, np.float32)], axis=1
    ).astype(np.float32)

    in2 = []
    for c in range(d.CORES):
        rs = slice(c * d.RPC, (c + 1) * d.RPC)
        in2.append({
            "hrf": hr_full,
            "gidx": np.ascontiguousarray(gidx[c]),
            "S": np.ascontiguousarray(S[c]),
            "hrT": res1[c]["hrT"],
            "ZT": res1[c]["ZT"],
            "HtxT": res1[c]["HtxT"],
            "hT32": _halves(hT[:, rs]).astype(np.float32),
            "Wc": W2,
            "bias": bias2,
        })
    res2 = _run(nc2, in2, d)

    out = np.empty((d.N, d.LAT), np.float32)
    h0 = np.empty((d.N, d.F), np.float32)
    for c in range(d.CORES):
        rs = slice(c * d.RPC, (c + 1) * d.RPC)
        out[rs] = res2[c]["outT"].T
        h0[rs, : d.FH] = res2[c]["h0T"][0].T
        h0[rs, d.FH :] = res2[c]["h0T"][1].T
    return out, h0


# revision 9
# speedup vs baseline: 1.0666x; 1.0232x over previous
"""GConvGRU (Chebyshev K=2 graph-conv GRU) + Linear head on 8 Trainium2 cores.

Strategy (graph/data parallel, per the node-partition sharding):
  - Nodes are partitioned contiguously across 8 cores (2500 rows each).
  - Only 3 sparse Laplacian applications are needed: lap(x), lap(h), lap(h*R)
    (the 6 ChebConvs share them).  Each is computed as, per destination block:
      gather source rows (dma_gather, fp16) -> one-hot matmul scatter into PSUM
    where the one-hot matrix S (norm-scaled, host-built) has edges on the
    partition (contraction) axis, so PSUM accumulates lap^T feature-major.
    x and h are interleaved per row so one descriptor fetches both.
  - Dense gate matmuls run feature-major: out^T[mo, n] += W-blocks against
    v^T tiles, N=500 per PSUM bank; interleaved with the scatter blocks so
    the PE fills the descriptor-generation shadow of the gathers (SWDGE
    descriptor generation, ~7.6 ns/edge on GpSimd, is the kernel bottleneck;
    everything else is scheduled to hide underneath it).
  - Per-(core,block) edge counts differ across cores while the NEFF is
    shared, so index streams are padded to a uniform chunk count with
    index 0 whose scatter coefficients are zero.
  - Two NEFF launches: L1 computes lap(x), lap(h), gates Z and R, the x-part
    of the candidate pre-activation, and h*R.  The host all-gathers h*R
    (transpose + concat), then L2 computes lap(h*R), Ht, h0 and the output
    head.  Cross-partition edges are handled by replicating the (read-only)
    gather sources to every core; random connectivity makes every core need
    nearly every row anyway.
"""

import sys

sys.path.insert(0, "/opt/trn_rl_repo")

from contextlib import ExitStack
from dataclasses import dataclass

import numpy as np

import concourse.bass as bass
import concourse.tile as tile
from concourse import bacc, mybir

F16 = mybir.dt.float16
F32 = mybir.dt.float32
I16 = mybir.dt.int16
AF = mybir.ActivationFunctionType


@dataclass(frozen=True)
class Dims:
    N: int = 20000        # nodes
    F: int = 256          # feature dim (= IN = HID)
    LAT: int = 128        # output dim of the linear head
    CORES: int = 8
    RPC: int = 2500       # rows per core
    DBLK: int = 125       # destination-block size for the scatter
    NCH: int = 500        # dense-matmul N chunk (<=512, fits one PSUM bank)

    @property
    def NBLK(self):
        return self.RPC // self.DBLK

    @property
    def NDCH(self):
        return self.RPC // self.NCH

    @property
    def BPC(self):  # scatter blocks per dense chunk
        return self.NCH // self.DBLK

    @property
    def FH(self):
        return self.F // 2


DIMS = Dims()

# ---------------------------------------------------------------------------
# NEFF builders
# ---------------------------------------------------------------------------


class _Scatter:
    """Emits the per-dst-block gather + one-hot scatter matmuls."""

    def __init__(self, nc, tc, ctx, d, cpb, src_dram, gidx_sb, S_dram,
                 lap_tiles, ew, gidx0_sb=None):
        self.nc, self.d, self.cpb = nc, d, cpb
        self.src_dram, self.gidx_sb, self.S_dram = src_dram, gidx_sb, S_dram
        self.gidx0_sb = gidx0_sb
        self.lap_tiles, self.ew = lap_tiles, ew
        self.nlt = len(lap_tiles)
        assert self.nlt * 128 == ew
        self.gpool = ctx.enter_context(tc.tile_pool(name="gpool", bufs=3))
        self.spool = ctx.enter_context(tc.tile_pool(name="spool", bufs=3))
        self.lppool = ctx.enter_context(
            tc.tile_pool(name="lappsum", bufs=1, space="PSUM"))
        self.offs = np.concatenate([[0], np.cumsum(cpb)]).astype(int)

    def block(self, b):
        nc, d, cpbb, off = self.nc, self.d, self.cpb[b], int(self.offs[b])
        ew = self.ew
        G = self.gpool.tile([128, cpbb * ew], F16, tag="G", name="G")
        if b == 0 and self.gidx0_sb is not None:
            idxs = self.gidx0_sb[:, 0 : 8 * cpbb]
        else:
            idxs = self.gidx_sb[:, 8 * off : 8 * (off + cpbb)]
        nc.gpsimd.dma_gather(
            out_ap=G[:].rearrange("p (c e) -> p c e", e=ew),
            in_ap=self.src_dram[:, :],
            idxs_ap=idxs,
            num_idxs=cpbb * 128,
            num_idxs_reg=cpbb * 128,
            elem_size=ew,
            single_packet=False,
        )
        Sb = self.spool.tile([128, cpbb * d.DBLK], F16, tag="S", name="S")
        nc.sync.dma_start(
            out=Sb[:], in_=self.S_dram[:, d.DBLK * off : d.DBLK * (off + cpbb)])
        psums = [self.lppool.tile([128, d.DBLK], F32, tag=f"lap{i}", name=f"lp{i}")
                 for i in range(self.nlt)]
        for j in range(cpbb):
            rhs = Sb[:, j * d.DBLK : (j + 1) * d.DBLK]
            for i in range(self.nlt):
                nc.tensor.matmul(
                    out=psums[i][:],
                    lhsT=G[:, j * ew + i * 128 : j * ew + i * 128 + 128],
                    rhs=rhs,
                    start=(j == 0),
                    stop=(j == cpbb - 1),
                )
        for i, lt in enumerate(self.lap_tiles):
            nc.vector.tensor_copy(
                out=lt[:, b * d.DBLK : (b + 1) * d.DBLK], in_=psums[i][:])


def build_l1(cpb, d=DIMS):
    """L1: lap(x), lap(h); Z = sigmoid(...); R = sigmoid(...); Htx partial; hR."""
    TCH = sum(cpb)
    nc = bacc.Bacc(
        "TRN2", target_bir_lowering=False, debug=False, enable_asserts=False,
        num_devices=d.CORES,
    )
    xh = nc.dram_tensor("xh", [d.N, 2 * d.F], F16, kind="ExternalInput").ap()
    gidx_d = nc.dram_tensor("gidx", [128, 8 * TCH], I16, kind="ExternalInput").ap()
    S_d = nc.dram_tensor("S", [128, TCH * d.DBLK], F16, kind="ExternalInput").ap()
    xT_d = nc.dram_tensor("xT", [2, 128, d.RPC], F16, kind="ExternalInput").ap()
    hT_d = nc.dram_tensor("hT", [2, 128, d.RPC], F16, kind="ExternalInput").ap()
    W_d = nc.dram_tensor("Wc", [128, 40 * 128], F16, kind="ExternalInput").ap()
    b_d = nc.dram_tensor("bias", [128, 4], F32, kind="ExternalInput").ap()
    ZT_d = nc.dram_tensor("ZT", [2, 128, d.RPC], F16, kind="ExternalOutput").ap()
    HtxT_d = nc.dram_tensor("HtxT", [2, 128, d.RPC], F32, kind="ExternalOutput").ap()
    hrT_d = nc.dram_tensor("hrT", [2, 128, d.RPC], F16, kind="ExternalOutput").ap()

    with tile.TileContext(nc) as tc, ExitStack() as ctx:
        cpool = ctx.enter_context(tc.tile_pool(name="const", bufs=1))
        # slot-0 indices ride the gather engine's own SWDGE queue (tiny, so
        # the first dma_gather isn't stuck behind the big HWDGE input batch)
        gidx0_sb = cpool.tile([128, 8 * cpb[0]], I16, tag="gidx0", name="gidx0")
        nc.gpsimd.dma_start(out=gidx0_sb[:], in_=gidx_d[:, 0 : 8 * cpb[0]])
        gidx_sb = cpool.tile([128, 8 * TCH], I16, tag="gidx", name="gidx")
        nc.sync.dma_start(out=gidx_sb[:, 8 * cpb[0] :], in_=gidx_d[:, 8 * cpb[0] :])
        lxT_sb, lhT_sb = (
            [cpool.tile([128, d.RPC], F16, tag=f"{nm}{kh}", name=f"{nm}{kh}")
             for kh in range(2)]
            for nm in ("lxT", "lhT")
        )
        sc = _Scatter(nc, tc, ctx, d, cpb, xh, gidx_sb, S_d,
                      [lxT_sb[0], lxT_sb[1], lhT_sb[0], lhT_sb[1]], 4 * 128,
                      gidx0_sb=gidx0_sb)

        apool = ctx.enter_context(tc.tile_pool(name="acc", bufs=3, space="PSUM"))
        stg = ctx.enter_context(tc.tile_pool(name="stg", bufs=3))

        for n in range(d.NDCH):
            for b in range(n * d.BPC, (n + 1) * d.BPC):
                sc.block(b)
            if n == 0:
                # resident dense operands, behind the first blocks' S loads
                W_sb = cpool.tile([128, 40 * 128], F16, tag="W", name="W")
                nc.sync.dma_start(out=W_sb[:], in_=W_d[:, :])
                bias_sb = cpool.tile([128, 4], F32, tag="bias", name="bias")
                nc.sync.dma_start(out=bias_sb[:], in_=b_d[:, :])
                xT_sb, hT_sb = (
                    [cpool.tile([128, d.RPC], F16, tag=f"{nm}{kh}", name=f"{nm}{kh}")
                     for kh in range(2)]
                    for nm in ("xT", "hT")
                )
                for kh in range(2):
                    nc.sync.dma_start(out=xT_sb[kh][:], in_=xT_d[kh])
                    nc.sync.dma_start(out=hT_sb[kh][:], in_=hT_d[kh])
                rhs4 = [xT_sb, lxT_sb, hT_sb, lhT_sb]
            nsl = slice(n * d.NCH, (n + 1) * d.NCH)
            # gate g: 0=z (sigmoid->ZT), 1=r (sigmoid->R, then hR), 2=htx (copy)
            for g, nt in ((0, 4), (1, 4), (2, 2)):
                for mh in range(2):
                    acc = apool.tile([128, d.NCH], F32, tag="acc", name="acc")
                    k = 0
                    for t in range(nt):
                        for kh in range(2):
                            wc = g * 16 + (t * 2 + kh) * 2 + mh
                            nc.tensor.matmul(
                                out=acc[:],
                                lhsT=W_sb[:, wc * 128 : (wc + 1) * 128],
                                rhs=rhs4[t][kh][:, nsl],
                                start=(k == 0),
                                stop=(k == nt * 2 - 1),
                            )
                            k += 1
                    if g == 0:
                        zc = stg.tile([128, d.NCH], F16, tag="zc", name="zc")
                        nc.scalar.activation(out=zc[:], in_=acc[:], func=AF.Sigmoid,
                                             bias=bias_sb[:, mh : mh + 1])
                        nc.sync.dma_start(out=ZT_d[mh][:, nsl], in_=zc[:])
                    elif g == 1:
                        rc = stg.tile([128, d.NCH], F16, tag="rc", name="rc")
                        nc.scalar.activation(out=rc[:], in_=acc[:], func=AF.Sigmoid,
                                             bias=bias_sb[:, 2 + mh : 3 + mh])
                        hrc = stg.tile([128, d.NCH], F16, tag="hrc", name="hrc")
                        nc.vector.tensor_mul(out=hrc[:], in0=hT_sb[mh][:, nsl],
                                             in1=rc[:])
                        nc.sync.dma_start(out=hrT_d[mh][:, nsl], in_=hrc[:])
                    else:
                        hc = stg.tile([128, d.NCH], F32, tag="hc", name="hc")
                        nc.vector.tensor_copy(out=hc[:], in_=acc[:])
                        nc.sync.dma_start(out=HtxT_d[mh][:, nsl], in_=hc[:])
    nc.compile()
    return nc


def build_l2(cpb, d=DIMS):
    """L2: lap(hR); Ht = tanh(Htx + hR@W + lap(hR)@W + b); h0; out head."""
    TCH = sum(cpb)
    nc = bacc.Bacc(
        "TRN2", target_bir_lowering=False, debug=False, enable_asserts=False,
        num_devices=d.CORES,
    )
    hrf = nc.dram_tensor("hrf", [d.N, d.F], F16, kind="ExternalInput").ap()
    gidx_d = nc.dram_tensor("gidx", [128, 8 * TCH], I16, kind="ExternalInput").ap()
    S_d = nc.dram_tensor("S", [128, TCH * d.DBLK], F16, kind="ExternalInput").ap()
    hrT_d = nc.dram_tensor("hrT", [2, 128, d.RPC], F16, kind="ExternalInput").ap()
    ZT_d = nc.dram_tensor("ZT", [2, 128, d.RPC], F16, kind="ExternalInput").ap()
    HtxT_d = nc.dram_tensor("HtxT", [2, 128, d.RPC], F32, kind="ExternalInput").ap()
    hT32_d = nc.dram_tensor("hT32", [2, 128, d.RPC], F32, kind="ExternalInput").ap()
    W_d = nc.dram_tensor("Wc", [128, 10 * 128], F16, kind="ExternalInput").ap()
    b_d = nc.dram_tensor("bias", [128, 3], F32, kind="ExternalInput").ap()
    h0T_d = nc.dram_tensor("h0T", [2, 128, d.RPC], F32, kind="ExternalOutput").ap()
    outT_d = nc.dram_tensor("outT", [128, d.RPC], F32, kind="ExternalOutput").ap()

    with tile.TileContext(nc) as tc, ExitStack() as ctx:
        cpool = ctx.enter_context(tc.tile_pool(name="const", bufs=1))
        gidx0_sb = cpool.tile([128, 8 * cpb[0]], I16, tag="gidx0", name="gidx0")
        nc.gpsimd.dma_start(out=gidx0_sb[:], in_=gidx_d[:, 0 : 8 * cpb[0]])
        gidx_sb = cpool.tile([128, 8 * TCH], I16, tag="gidx", name="gidx")
        nc.sync.dma_start(out=gidx_sb[:, 8 * cpb[0] :], in_=gidx_d[:, 8 * cpb[0] :])
        lhrT_sb = [cpool.tile([128, d.RPC], F16, tag=f"lhrT{kh}", name=f"lhrT{kh}")
                   for kh in range(2)]
        sc = _Scatter(nc, tc, ctx, d, cpb, hrf, gidx_sb, S_d,
                      [lhrT_sb[0], lhrT_sb[1]], 2 * 128, gidx0_sb=gidx0_sb)

        apool = ctx.enter_context(tc.tile_pool(name="acc", bufs=3, space="PSUM"))
        stg = ctx.enter_context(tc.tile_pool(name="stg", bufs=3))

        for n in range(d.NDCH):
            for b in range(n * d.BPC, (n + 1) * d.BPC):
                sc.block(b)
            if n == 0:
                W_sb = cpool.tile([128, 10 * 128], F16, tag="W", name="W")
                nc.sync.dma_start(out=W_sb[:], in_=W_d[:, :])
                bias_sb = cpool.tile([128, 3], F32, tag="bias", name="bias")
                nc.sync.dma_start(out=bias_sb[:], in_=b_d[:, :])
                hrT_sb, ZT_sb = (
                    [cpool.tile([128, d.RPC], F16, tag=f"{nm}{kh}", name=f"{nm}{kh}")
                     for kh in range(2)]
                    for nm in ("hrT", "ZT")
                )
                HtxT_sb, hT32_sb = (
                    [cpool.tile([128, d.RPC], F32, tag=f"{nm}{kh}", name=f"{nm}{kh}")
                     for kh in range(2)]
                    for nm in ("HtxT", "hT32")
                )
                for kh in range(2):
                    nc.sync.dma_start(out=hrT_sb[kh][:], in_=hrT_d[kh])
                    nc.sync.dma_start(out=ZT_sb[kh][:], in_=ZT_d[kh])
                    nc.sync.dma_start(out=HtxT_sb[kh][:], in_=HtxT_d[kh])
                    nc.sync.dma_start(out=hT32_sb[kh][:], in_=hT32_d[kh])
                rhs2 = [hrT_sb, lhrT_sb]
            nsl = slice(n * d.NCH, (n + 1) * d.NCH)
            relu_c = []
            for mh in range(2):
                acc = apool.tile([128, d.NCH], F32, tag="acc", name="acc")
                k = 0
                for t in range(2):
                    for kh in range(2):
                        wc = (t * 2 + kh) * 2 + mh
                        nc.tensor.matmul(
                            out=acc[:],
                            lhsT=W_sb[:, wc * 128 : (wc + 1) * 128],
                            rhs=rhs2[t][kh][:, nsl],
                            start=(k == 0), stop=(k == 3),
                        )
                        k += 1
                nc.vector.tensor_add(out=acc[:], in0=acc[:],
                                     in1=HtxT_sb[mh][:, nsl])
                ht = stg.tile([128, d.NCH], F32, tag="ht", name="ht")
                nc.scalar.activation(out=ht[:], in_=acc[:], func=AF.Tanh,
                                     bias=bias_sb[:, mh : mh + 1])
                # h0 = Ht + Z*(h - Ht)
                z32 = stg.tile([128, d.NCH], F32, tag="z32", name="z32")
                nc.vector.tensor_copy(out=z32[:], in_=ZT_sb[mh][:, nsl])
                dd = stg.tile([128, d.NCH], F32, tag="dd", name="dd")
                nc.vector.tensor_sub(out=dd[:], in0=hT32_sb[mh][:, nsl], in1=ht[:])
                nc.vector.tensor_mul(out=dd[:], in0=dd[:], in1=z32[:])
                h0c = stg.tile([128, d.NCH], F32, tag="h0c", name="h0c")
                nc.vector.tensor_add(out=h0c[:], in0=ht[:], in1=dd[:])
                nc.sync.dma_start(out=h0T_d[mh][:, nsl], in_=h0c[:])
                rl = stg.tile([128, d.NCH], F16, tag="rl", name="rl")
                nc.vector.tensor_scalar_max(rl[:], h0c[:], 0.0)
                relu_c.append(rl)
            acc = apool.tile([128, d.NCH], F32, tag="acc", name="acc")
            for kh in range(2):
                nc.tensor.matmul(
                    out=acc[:], lhsT=W_sb[:, (8 + kh) * 128 : (9 + kh) * 128],
                    rhs=relu_c[kh][:], start=(kh == 0), stop=(kh == 1),
                )
            oc = stg.tile([128, d.NCH], F32, tag="oc", name="oc")
            nc.vector.tensor_scalar_add(oc[:], acc[:], bias_sb[:, 2:3])
            nc.sync.dma_start(out=outT_d[:, nsl], in_=oc[:])
    nc.compile()
    return nc


# ---------------------------------------------------------------------------
# Host-side preprocessing
# ---------------------------------------------------------------------------


def _wrap_idx(idx_arr):
    """Edge-index array -> dma_gather wrapped layout [128, len/16] int16."""
    a = idx_arr.reshape(-1, 16).T  # [16, n/16]
    return np.tile(a, (8, 1))


def prep_edges(edge_index, edge_weight, d=DIMS):
    """Partition/sort/pad edges; build per-core gather-index + scatter tensors."""
    src = edge_index[0].astype(np.int64)
    dst = edge_index[1].astype(np.int64)
    w = edge_weight.astype(np.float32)
    deg = np.zeros(d.N, np.float32)
    np.add.at(deg, src, w)
    dis = np.where(deg > 0, 1.0 / np.sqrt(np.where(deg > 0, deg, 1.0)), 0.0).astype(
        np.float32
    )
    norm = (-dis[src] * w * dis[dst]).astype(np.float32)

    core = dst // d.RPC
    blk = (dst % d.RPC) // d.DBLK
    dloc = (dst % d.RPC) % d.DBLK
    counts = np.bincount(core * d.NBLK + blk, minlength=d.CORES * d.NBLK).reshape(
        d.CORES, d.NBLK
    )
    # Per-core block->slot permutation (largest block first): slot s then
    # holds similarly-sized blocks on every core, so the per-slot max that
    # sizes the shared gather stream wastes far less padding than the
    # natural block order would.
    perm = np.argsort(-counts, axis=1, kind="stable")  # [CORES, NBLK]: slot->blk
    invperm = np.argsort(perm, axis=1)                 # blk->slot
    slot = invperm[core, blk]
    order = np.lexsort((src, slot, core))
    src_s, norm_s, dloc_s = src[order], norm[order], dloc[order]
    counts_slot = np.take_along_axis(counts, perm, axis=1)
    cpb = tuple(
        max(1, int(np.ceil(counts_slot[:, s].max() / 128))) for s in range(d.NBLK)
    )
    TCH = sum(cpb)
    starts = np.zeros(d.CORES * d.NBLK + 1, np.int64)
    np.cumsum(counts_slot.reshape(-1), out=starts[1:])

    # pad with index 0 (valid row; its scatter coefficients stay 0)
    gidx = np.zeros((d.CORES, 128, 8 * TCH), np.int16)
    S = np.zeros((d.CORES, 128, TCH * d.DBLK), np.float16)
    for c in range(d.CORES):
        off = 0
        for s in range(d.NBLK):
            cnt = counts_slot[c, s]
            s0 = starts[c * d.NBLK + s]
            idx_arr = np.zeros(cpb[s] * 128, np.int16)
            idx_arr[:cnt] = src_s[s0 : s0 + cnt]
            gidx[c, :, 8 * off : 8 * (off + cpb[s])] = _wrap_idx(idx_arr)
            p = np.arange(cnt)
            S[c, p % 128, d.DBLK * off + (p // 128) * d.DBLK + dloc_s[s0 : s0 + cnt]] = (
                norm_s[s0 : s0 + cnt]
            )
            off += cpb[s]
    # pcol[c, j]: permuted column of natural local node j
    j = np.arange(d.RPC)
    pcol = invperm[:, j // d.DBLK] * d.DBLK + (j % d.DBLK)  # [CORES, RPC]
    return cpb, gidx, S, pcol


def _pack_w(mats):
    """mats: list of [256,256]-ish fp32 -> [128, nblocks*128] fp16 lhsT blocks."""
    blocks = []
    for m in mats:
        K, M = m.shape
        for kh in range(K // 128):
            for mh in range(M // 128):
                blocks.append(m[kh * 128 : (kh + 1) * 128, mh * 128 : (mh + 1) * 128])
    return np.concatenate(blocks, axis=1).astype(np.float16)


def _halves(vT):
    """[F, R] -> [2, 128, R] contiguous."""
    F = vT.shape[0]
    return np.ascontiguousarray(vT.reshape(2, F // 2, -1))


# ---------------------------------------------------------------------------
# kernel() entry point
# ---------------------------------------------------------------------------

_CACHE = {}
LAST_EXEC_NS = []


def _get_neffs(cpb, d):
    key = (cpb, d)
    if key not in _CACHE:
        _CACHE[key] = (build_l1(cpb, d), build_l2(cpb, d))
    return _CACHE[key]


def _run(nc, in_maps, d):
    from concourse.bass_utils import run_bass_kernel_spmd

    res = run_bass_kernel_spmd(nc, in_maps, core_ids=list(range(d.CORES)))
    if res.exec_time_ns is not None:
        LAST_EXEC_NS.append(res.exec_time_ns)
    return res.results


def kernel(
    x, h, edge_index, edge_weight,
    Wxz, bxz, Whz, bhz, Wxr, bxr, Whr, bhr, Wxh, bxh, Whh, bhh, Wl, bl,
    dims=DIMS,
):
    d = dims
    LAST_EXEC_NS.clear()
    x = np.asarray(x, np.float32)
    h = np.asarray(h, np.float32)
    cpb, gidx, S, pcol = prep_edges(
        np.asarray(edge_index), np.asarray(edge_weight), d)
    nc1, nc2 = _get_neffs(cpb, d)
    # invp[c]: permuted-column -> natural-column (so natural[:, invp] is the
    # slot-permuted layout the NEFF works in)
    invp = np.argsort(pcol, axis=1)

    xh = np.concatenate([x, h], axis=1).astype(np.float16)
    xT = np.ascontiguousarray(x.T)
    hT = np.ascontiguousarray(h.T)

    W1 = _pack_w([
        np.asarray(Wxz[0]), np.asarray(Wxz[1]), np.asarray(Whz[0]), np.asarray(Whz[1]),
        np.asarray(Wxr[0]), np.asarray(Wxr[1]), np.asarray(Whr[0]), np.asarray(Whr[1]),
        np.asarray(Wxh[0]), np.asarray(Wxh[1]),
    ])
    bz = (np.asarray(bxz) + np.asarray(bhz)).astype(np.float32)
    br = (np.asarray(bxr) + np.asarray(bhr)).astype(np.float32)
    bias1 = np.stack(
        [bz[: d.FH], bz[d.FH :], br[: d.FH], br[d.FH :]], axis=1
    ).astype(np.float32)

    in1 = []
    for c in range(d.CORES):
        rs = slice(c * d.RPC, (c + 1) * d.RPC)
        in1.append({
            "xh": xh,
            "gidx": np.ascontiguousarray(gidx[c]),
            "S": np.ascontiguousarray(S[c]),
            "xT": _halves(xT[:, rs][:, invp[c]]).astype(np.float16),
            "hT": _halves(hT[:, rs][:, invp[c]]).astype(np.float16),
            "Wc": W1,
            "bias": bias1,
        })
    res1 = _run(nc1, in1, d)

    # host exchange: assemble full h*R, row-major fp16 (un-permute columns)
    hr_full = np.empty((d.N, d.F), np.float16)
    for c in range(d.CORES):
        hrT_c = res1[c]["hrT"]  # [2, 128, RPC], slot-permuted columns
        rs = slice(c * d.RPC, (c + 1) * d.RPC)
        hr_full[rs, : d.FH] = hrT_c[0][:, pcol[c]].T
        hr_full[rs, d.FH :] = hrT_c[1][:, pcol[c]].T

    W2 = _pack_w([np.asarray(Whh[0]), np.asarray(Whh[1]), np.asarray(Wl)])
    bht = (np.asarray(bxh) + np.asarray(bhh)).astype(np.float32)
    bias2 = np.stack(
        [bht[: d.FH], bht[d.FH :], np.asarray(bl, np.float32)], axis=1
    ).astype(np.float32)

    in2 = []
    for c in range(d.CORES):
        rs = slice(c * d.RPC, (c + 1) * d.RPC)
        in2.append({
            "hrf": hr_full,
            "gidx": np.ascontiguousarray(gidx[c]),
            "S": np.ascontiguousarray(S[c]),
            "hrT": res1[c]["hrT"],
            "ZT": res1[c]["ZT"],
            "HtxT": res1[c]["HtxT"],
            "hT32": _halves(hT[:, rs][:, invp[c]]).astype(np.float32),
            "Wc": W2,
            "bias": bias2,
        })
    res2 = _run(nc2, in2, d)

    out = np.empty((d.N, d.LAT), np.float32)
    h0 = np.empty((d.N, d.F), np.float32)
    for c in range(d.CORES):
        rs = slice(c * d.RPC, (c + 1) * d.RPC)
        out[rs] = res2[c]["outT"][:, pcol[c]].T
        h0[rs, : d.FH] = res2[c]["h0T"][0][:, pcol[c]].T
        h0[rs, d.FH :] = res2[c]["h0T"][1][:, pcol[c]].T
    return out, h0
